# revision 38
# baseline (speedup 1.0000x reference)
"""Divergence-free RBF kernel Gram matrix on 8 Trainium2 NeuronCores.

Math: for d=2, with scaled coords x' = x*exp(-ll/2):
  dx = x0_i - y0_j, dy = x1_i - y1_j, r2 = dx^2 + dy^2, e = exp(-r2/2)
  K[2i+0, 2j+0] = e * (1 - dy^2)          (channel A)
  K[2i+0, 2j+1] = K[2i+1, 2j+0] = e*dx*dy (channel X)
  K[2i+1, 2j+1] = e * (1 - dx^2)          (channel C)

Each polynomial factor is low-rank in the basis {1, x0, x1, x0*x1, x0^2,
x1^2}: host precomputes X-side rows L and Y-side coefficient columns; the
device builds A/X/C/r2 with PE matmuls (bf16 hi/lo 3-term, K=18), exp on ACT,
and one multiply-by-e per output element. Output is written bf16 (rel-err
budget 2e-2; this lands ~1.5e-3), halving HBM write traffic - the dominant
cost in this memory-bound problem.

PE trick: K=18 uses only 18 of 128 PE rows. The four matmuls of each 512-j
group (r2, C, and two A/X-pair halves) are issued to four different 32-row
tile_positions, whose moving data lives on four different SBUF partition
bands - the PE executes same-FIFO matmuls to distinct row groups
CONCURRENTLY, so a jgroup's PE time is ~1 matmul, not 4.

Consumers per jgroup, PSUM tiles [A,X pairs | C | r2]: ACT does exp(r2), DVE
multiplies the (A,X) block by e (broadcast over the pair dim), the C-plane
multiply alternates DVE-direct vs ACT-copy+GpSimd-multiply (GpSimd cannot
read PSUM), and the duplicate X column of the odd output row is an ACT SBUF
copy.

Sharding: rows of X (n axis) split across 8 cores, 512 each -> each core
writes 1024 rows of the (8192, 8192) Gram matrix. No communication.
"""

import numpy as np
import ml_dtypes

N = 4096          # X rows
M = 4096          # Y rows
NCORES = 8
NPC = N // NCORES  # 512 X rows per core
IB = 128           # i-block = partition count
NIB = NPC // IB    # 4 i-blocks per core
JG = 512           # j's per jgroup
NJG = M // JG      # 8 jgroups (per i-block)
KST = 18           # stacked contraction rows (3 x 6 basis)

_cache = {}


# C-plane multiply on DVE for ~5/8 of jgroups, else ACT-copy + GpSimd
# multiply (GpSimd runs ONLY tensor_mul - mixing op types on Q7 triggers
# ~10us microcode library reloads). DVE route for the final groups to cut
# the tail.
def _c_on_dve(gg):
    return gg % 8 < 5 or gg >= 29


def _hi_lo(a, dt):
    hi = a.astype(dt)
    lo = (a - hi.astype(np.float64)).astype(dt)
    return hi, lo


def _prepare_inputs(X, Y, log_length_scale):
    bf = ml_dtypes.bfloat16
    s = float(np.exp(-0.5 * np.float64(np.asarray(log_length_scale).reshape(-1)[0])))
    xs = np.asarray(X, dtype=np.float64).reshape(N, 2) * s
    ys = np.asarray(Y, dtype=np.float64).reshape(M, 2) * s
    x0, x1 = xs[:, 0], xs[:, 1]
    y0, y1 = ys[:, 0], ys[:, 1]
    one_n, zero_m, one_m = np.ones(N), np.zeros(M), np.ones(M)

    L = np.stack([one_n, x0, x1, x0 * x1, x0 ** 2, x1 ** 2])
    Lh, Ll = _hi_lo(L, bf)
    L18 = np.concatenate([Lh, Ll, Lh], axis=0)  # pairs with [Rh;Rh;Rl]

    c_00 = np.stack([1 - y1 ** 2, zero_m, 2 * y1, zero_m, zero_m, -one_m])
    c_dxdy = np.stack([y0 * y1, -y1, -y0, one_m, zero_m, zero_m])
    c_11 = np.stack([1 - y0 ** 2, 2 * y0, zero_m, zero_m, -one_m, zero_m])
    c_r2 = np.stack([y0 ** 2 + y1 ** 2, -2 * y0, -2 * y1, zero_m, one_m, one_m])

    def r18(c):
        ch, cl = _hi_lo(c, bf)
        return np.concatenate([ch, ch, cl], axis=0).astype(np.float64)

    R_A, R_X, R_C, R_R = r18(c_00), r18(c_dxdy), r18(c_11), r18(c_r2)

    # Stationary, replicated into the four 32-partition bands.
    wts = np.zeros((128, N), bf)
    for b in range(4):
        wts[32 * b:32 * b + KST, :] = L18

    # Moving tensor, one channel per band so each jgroup's 4 matmuls hit 4
    # different PE row groups (concurrent execution):
    #   band 0: C plane, band 1: r2 plane,
    #   band 2: (A,X) pairs j0..255 of each group, band 3: pairs j256..511.
    jidx = np.arange(M).reshape(NJG, JG)
    ax = np.zeros((KST, NJG, 2 * JG))
    ax[:, :, 0::2] = R_A[:, jidx]
    ax[:, :, 1::2] = R_X[:, jidx]
    rhs = np.zeros((128, 4096), bf)
    rhs[0:KST, :] = R_C[:, jidx].reshape(KST, 4096)
    rhs[32:32 + KST, :] = R_R[:, jidx].reshape(KST, 4096)
    rhs[64:64 + KST, :] = ax[:, :, 0:512].reshape(KST, 4096)
    rhs[96:96 + KST, :] = ax[:, :, 512:1024].reshape(KST, 4096)

    return wts, np.ascontiguousarray(rhs)


def _build_module(bass_cls=None, **bass_kw):
    from concourse import bacc, mybir
    import concourse.tile as tile

    bf16 = mybir.dt.bfloat16
    f32 = mybir.dt.float32
    Exp = mybir.ActivationFunctionType.Exp

    if bass_cls is None:
        bass_cls = bacc.Bacc
    nc = bass_cls("TRN2", target_bir_lowering=False, debug=False,
                  enable_asserts=False, **bass_kw)
    wts_d = nc.dram_tensor("wts", [128, NPC], bf16, kind="ExternalInput")
    rhs_d = nc.dram_tensor("rhs", [128, 4096], bf16, kind="ExternalInput")
    out_d = nc.dram_tensor("out", [2 * NPC, 2 * M], bf16, kind="ExternalOutput")

    with tile.TileContext(nc) as tc:
        with (
            tc.tile_pool(name="const", bufs=1) as cpool,
            tc.tile_pool(name="outp", bufs=2) as opool,
            tc.tile_pool(name="ep", bufs=3) as epool,
            tc.tile_pool(name="scp", bufs=3) as scpool,
            tc.tile_pool(name="ps", bufs=2, space="PSUM") as ppool,
        ):
            wts_sb = cpool.tile([128, NPC], bf16)
            rhs_sb = cpool.tile([128, 4096], bf16)
            # parallel-issue input loads from different engine queues;
            # chunked by jgroup pair so jgroup 0 can start ~1us in
            nc.scalar.dma_start(out=wts_sb[:], in_=wts_d[:, :])
            nc.sync.dma_start(out=rhs_sb[:, 0:512], in_=rhs_d[:, 0:512])
            nc.sync.dma_start(out=rhs_sb[:, 512:1024], in_=rhs_d[:, 512:1024])
            nc.sync.dma_start(out=rhs_sb[:, 1024:2048], in_=rhs_d[:, 1024:2048])
            nc.sync.dma_start(out=rhs_sb[:, 2048:3072], in_=rhs_d[:, 2048:3072])
            nc.sync.dma_start(out=rhs_sb[:, 3072:4096], in_=rhs_d[:, 3072:4096])

            out_view = out_d.ap().rearrange("(i t) c -> i t c", t=2)

            for ib in range(NIB):
                out_all = opool.tile([IB, 4 * M], bf16, tag="out_all")
                i0 = ib * IB
                last_ib = ib == NIB - 1

                # X-duplicate for odd (ACT-route) jgroups, emitted one jgroup
                # late: it reads the pair-mul's output, so putting it at the
                # ACT queue head right away would block exp(g+1) behind a DVE
                # dependency.
                def x_copy(g):
                    out0 = out_all[:, g * 1024:(g + 1) * 1024].rearrange(
                        "p (j t) -> p j t", t=2)
                    h1 = out_all[:, 8192 + g * 1024:8192 + (g + 1) * 1024]
                    h1 = h1.rearrange("p (j t) -> p j t", t=2)
                    nc.scalar.copy(h1[:, :, 0:1].squeeze(2),
                                   out0[:, :, 1:2].squeeze(2))

                def flush_dma(g):
                    # h0/h1 rows of jgroup pair (g-1, g); h1 DMA waits on the
                    # x_copy and C-multiply of both groups
                    w = 1024 if last_ib else 2048
                    cb = g * 1024 if last_ib else (g - 1) * 1024
                    nc.scalar.dma_start(
                        out=out_view[i0:i0 + IB, 0:1, cb:cb + w].squeeze(1),
                        in_=out_all[:, cb:cb + w])
                    nc.sync.dma_start(
                        out=out_view[i0:i0 + IB, 1:2, cb:cb + w].squeeze(1),
                        in_=out_all[:, 8192 + cb:8192 + cb + w])

                for g in range(NJG):
                    gg = ib * NJG + g

                    def mm(out, band):
                        wt = wts_sb[32 * band:32 * band + KST,
                                    ib * IB:(ib + 1) * IB]
                        rh = rhs_sb[32 * band:32 * band + KST,
                                    g * JG:(g + 1) * JG]
                        nc.tensor.matmul(out, wt, rh, start=True, stop=True,
                                         tile_position=(32 * band, 0))

                    # separate 1-bank PSUM tiles: finer WAR deps (the r2
                    # plane of jgroup g+2 only waits on exp(g), not on the
                    # slower C-multiply chain)
                    mr = ppool.tile([IB, 512], f32, tag="memo_r")
                    mc = ppool.tile([IB, 512], f32, tag="memo_c")
                    max_ = ppool.tile([IB, 1024], f32, tag="memo_ax")
                    mm(mr[:], 1)              # r2 plane
                    mm(mc[:], 0)              # C plane
                    ebig = epool.tile([IB, JG], f32, tag="e")
                    nc.scalar.activation(ebig[:], mr[:], Exp, scale=-0.5)
                    mm(max_[:, 0:512], 2)     # (A,X) pairs, first half
                    mm(max_[:, 512:1024], 3)  # second half

                    out0 = out_all[:, g * 1024:(g + 1) * 1024].rearrange(
                        "p (j t) -> p j t", t=2)
                    in0 = max_[:].rearrange("p (j t) -> p j t", t=2)
                    eb = ebig[:].unsqueeze(2).broadcast_to([IB, JG, 2])
                    h1 = out_all[:, 8192 + g * 1024:8192 + (g + 1) * 1024]
                    h1 = h1.rearrange("p (j t) -> p j t", t=2)
                    h1o = h1[:, :, 1:2].squeeze(2)

                    final = gg == NIB * NJG - 1
                    if final:
                        # last group: C multiply first so the kernel's tail
                        # chain is pair-mul -> x_copy -> DMA only
                        nc.vector.tensor_mul(h1o, mc[:], ebig[:])
                    nc.vector.tensor_mul(out0, in0, eb)  # h0 = (A,X) * e
                    if final:
                        pass
                    elif _c_on_dve(gg):
                        nc.vector.tensor_mul(h1o, mc[:], ebig[:])
                    else:
                        sc = scpool.tile([IB, JG], bf16, tag="sc")
                        nc.scalar.copy(sc[:], mc[:])
                        nc.gpsimd.tensor_mul(h1o, sc[:], ebig[:])

                    if g > 0:
                        x_copy(g - 1)
                        if last_ib or g % 2 == 0:
                            flush_dma(g - 1)
                x_copy(NJG - 1)
                flush_dma(NJG - 1)
    nc.finalize()
    return nc


def _run(X, Y, log_length_scale, trace=False):
    from concourse.bass_utils import run_bass_kernel_spmd

    wts, rhs = _prepare_inputs(X, Y, log_length_scale)
    if "nc" not in _cache:
        _cache["nc"] = _build_module()
    nc = _cache["nc"]
    in_maps = [
        {
            "wts": np.ascontiguousarray(wts[:, c * NPC:(c + 1) * NPC]),
            "rhs": rhs,
        }
        for c in range(NCORES)
    ]
    res = run_bass_kernel_spmd(nc, in_maps, core_ids=list(range(NCORES)),
                               trace=trace)
    out = np.concatenate([r["out"].astype(np.float32) for r in res.results],
                         axis=0)
    return out.reshape(1, 2 * N, 2 * M), res


def kernel(X, Y, log_length_scale):
    out, _ = _run(np.asarray(X), np.asarray(Y), np.asarray(log_length_scale))
    return out
